# revision 29
# baseline (speedup 1.0000x reference)
"""Differentiable OMP (top-k masking) Trainium2 kernel.

Strategy (pure data parallelism over batch, 8 batches/core on 8 cores):
  The straight-through softmax terms cancel numerically in the forward pass,
  so each OMP iteration reduces to:
    pd    = proj0 - nzW @ G[idx_sel, :]        (argmax drive)
    idx_i = argmax |pd|
    solve (G[S,S] + reg I) nzW = proj0[S] incrementally (bordered inverse,
    rank-one product form) -- all O(i^2) work batched on 8 partitions.
  where G = D^T D (Gram of the shared dictionary) and proj0 = y @ D are
  computed once on device.  The final reconstruction gathers the 32 selected
  dictionary columns per batch from the X shard and combines with nzW on the
  tensor engine.  Only ~5 MB of the 32 MB X shard is ever read (indirect
  DMA gather with on-device indices).
"""

import os
import sys

for _p in ("/opt/trn_rl_repo", "/root/.axon_site/_ro/trn_rl_repo"):
    if os.path.isdir(_p) and _p not in sys.path:
        sys.path.insert(0, _p)

import numpy as np

import concourse.bass as bass
import concourse.mybir as mybir
import concourse.tile as tile
from concourse.bass_utils import run_bass_kernel_spmd
from concourse.masks import make_identity
from concourse.vector_clock import ScopedClock

F32 = mybir.dt.float32
U32 = mybir.dt.uint32
OP = mybir.AluOpType
AF = mybir.ActivationFunctionType
AX = mybir.AxisListType

NCORES = 8
NB = 8            # batches per core
L = 1024          # signal length
NA = 1025         # atoms (1024 + bias column)
K = 32            # n_nonzero_coefs
REG = float(np.log1p(np.exp(np.float32(-5.0), dtype=np.float32), dtype=np.float32))
GBUF_ROWS = NA + NB          # G rows then proj0 rows
NCHUNKS = [(0, 512), (512, 512), (1024, 1)]


_PATCHED = False


def _patch_tile_drain():
    """This walrus build rejects >1 sync waits per instruction: split the
    final-drain waits onto SP nops, and split any lowered instruction's
    extra waits onto same-engine nops."""
    global _PATCHED
    if _PATCHED:
        return
    _PATCHED = True

    _orig_commit_and_lower = tile.TileContext._commit_and_lower

    def _commit_and_lower_split(self, inst, original_block, old_bb_map, bb_to_exit):
        si = getattr(inst, "sync_info", None)
        if si is not None and si.on_wait and len(si.on_wait) > 1:
            waits = list(si.on_wait)
            for j, w in enumerate(waits[1:]):
                nop = mybir.InstNoOp(
                    name=f"{inst.name}-wsplit{j}", ins=[], outs=[], engine=inst.engine
                )
                nop.sync_info = mybir.SyncInfo(on_wait=[w], on_update=[])
                _orig_commit_and_lower(self, nop, original_block, old_bb_map, bb_to_exit)
            inst.sync_info = mybir.SyncInfo(
                on_wait=[waits[0]],
                on_update=list(si.on_update) if si.on_update else [],
            )
        return _orig_commit_and_lower(self, inst, original_block, old_bb_map, bb_to_exit)

    def _drain_and_barrier_split(self, tick_clock, wait_clock):
        nc = self.nc
        drain_inst = nc.sync.drain()
        wait_clock.add_sem_waits(
            drain_inst.ins, ScopedClock({None: tick_clock.global_clock})
        )
        si = drain_inst.ins.sync_info
        waits = list(si.on_wait) if si is not None and si.on_wait else []
        if len(waits) > 1:
            drain_inst.ins.sync_info = mybir.SyncInfo(
                on_wait=[waits[0]],
                on_update=list(si.on_update) if si.on_update else [],
            )
            for w in waits[1:]:
                n = nc.sync.nop()
                n.ins.sync_info = mybir.SyncInfo(on_wait=[w], on_update=[])

        nc.all_engine_barrier()
        assert self.sems is not None
        popped = nc._tile_sem_poison_stack.pop()
        assert popped is self._sem_poison
        nc.clear_and_free_semaphores(list(self.sems.allocated().values()))
        nc.all_engine_barrier()

    tile.TileContext._drain_and_barrier = _drain_and_barrier_split
    tile.TileContext._commit_and_lower = _commit_and_lower_split


def _cdiv(a, b):
    return (a + b - 1) // b


def _build_program():
    _patch_tile_drain()
    nc = bass.Bass()

    d_mat = nc.dram_tensor("d_mat", [L, NA], F32, kind="ExternalInput")
    y_t = nc.dram_tensor("y_t", [L, NB], F32, kind="ExternalInput")
    xt_pad = nc.dram_tensor("xt_pad", [NB * NA, L], F32, kind="ExternalInput")
    rhs_init = nc.dram_tensor("rhs_init", [128, 3, NB], F32, kind="ExternalInput")
    bdmask_in = nc.dram_tensor("bdmask_in", [128, 3, NB, K], F32, kind="ExternalInput")
    negsel_in = nc.dram_tensor("negsel_in", [NB, 3, 128], F32, kind="ExternalInput")
    wmask_in = nc.dram_tensor("wmask_in", [128, 2, NB, K], F32, kind="ExternalInput")
    betabase_in = nc.dram_tensor("betabase_in", [NB, 1], F32, kind="ExternalInput")
    xbase_in = nc.dram_tensor("xbase_in", [NB, 1], F32, kind="ExternalInput")
    # partition-expansion helpers (indirect DMA wants one index per partition)
    gselx_in = nc.dram_tensor("gselx_in", [NB, 128], F32, kind="ExternalInput")
    backsel_in = nc.dram_tensor("backsel_in", [128, NB], F32, kind="ExternalInput")
    jmask_in = nc.dram_tensor("jmask_in", [128, 3, K + 2], F32, kind="ExternalInput")
    bsel16_in = nc.dram_tensor("bsel16_in", [NB, 128], F32, kind="ExternalInput")
    rjmask_in = nc.dram_tensor("rjmask_in", [128, 2, K], F32, kind="ExternalInput")
    out_r = nc.dram_tensor("out_r", [NB, L], F32, kind="ExternalOutput")
    gbuf = nc.dram_tensor("gbuf", [GBUF_ROWS * NA, 1], F32, kind="Internal")

    gflat_ap = gbuf[:, :]                                            # element gather
    grows_ap = gbuf[:, :].rearrange("(r c) x -> r (c x)", c=NA)      # row gather

    import contextlib

    with tile.TileContext(nc) as tc, contextlib.ExitStack() as ctx:
        st = ctx.enter_context(tc.tile_pool(name="st", bufs=1))
        pp = ctx.enter_context(tc.tile_pool(name="pp", bufs=1, space="PSUM"))

        # ---------------- persistent state ----------------
        dsb = st.tile([128, 8, NA], F32)          # D, L split in 8 chunks
        ysb = st.tile([128, 8, NB], F32)          # y^T
        gstage = st.tile([128, 9, NA], F32)       # G staging (rows m*128+p)
        bigt = st.tile([128, 3, NA], F32)         # K-rows: proj0(8) + G rows
        rhs_t = st.tile([128, 3, NB], F32)        # matmul weights per K-row
        rhsi = st.tile([128, 3, NB], F32)         # inject-row pattern (ID8)
        rhsr = st.tile([128, NB], F32)            # rebuild scratch
        bdmask = st.tile([128, 3, NB, K], F32)
        wmask = st.tile([128, 2, NB, K], F32)
        negsel = st.tile([NB, 3, 128], F32)
        betabase = st.tile([NB, 1], F32)
        xbase = st.tile([NB, 1], F32)
        id128 = st.tile([128, 128], F32)
        gselx = st.tile([NB, 128], F32)
        backsel = st.tile([128, NB], F32)
        jmask = st.tile([128, 3, K + 2], F32)
        bsel16 = st.tile([NB, 128], F32)
        rjmask = st.tile([128, 2, K], F32)
        prodg = st.tile([128, K + 2], F32)
        fillr = st.tile([NB, 512], mybir.dt.bfloat16)   # HAM warm-keeper rhs
        prod2_t = st.tile([128, 3, NB, K], F32)
        rhsr2 = st.tile([128, 3, NB], F32)
        offsP = st.tile([128, 3], F32)
        offsPu = st.tile([128, 3], U32)
        gsmP = st.tile([128, 3], F32)
        grhs = st.tile([128, 3, K + 2], F32)
        xoffP = st.tile([128, 2], F32)
        xoffPu = st.tile([128, 2], U32)
        prodr = st.tile([128, K], F32)

        vmat = st.tile([NB, K, K], F32)
        nzw = st.tile([NB, K], F32)
        sinv_v = st.tile([NB, K], F32)
        bvec = st.tile([NB, K], F32)
        idxm = st.tile([NB, K], F32)
        gsm = st.tile([NB, K + 2], F32)
        offs = st.tile([NB, K + 2], F32)
        idxf = st.tile([NB, 1], F32)
        u_t = st.tile([NB, K], F32)
        c_t = st.tile([NB, K], F32)
        ct_t = st.tile([NB, K], F32)
        tmp3 = st.tile([NB, K, K], F32)
        tmp4 = st.tile([NB, K, K], F32)
        tmp5 = st.tile([NB, K], F32)
        sdot = st.tile([NB, 1], F32)
        s_t = st.tile([NB, 1], F32)
        alpha = st.tile([NB, 1], F32)
        ubdot = st.tile([NB, 1], F32)
        pdabs = st.tile([NB, NA], F32)
        mx8 = st.tile([NB, 8], F32)
        mi8 = st.tile([NB, 8], U32)
        prod_t = st.tile([128, NB, K], F32)
        xoff = st.tile([NB, K], F32)
        xsel = st.tile([128, 2, L], F32)
        wsel = st.tile([128, 2, NB], F32)
        outsb = st.tile([NB, L], F32)
        p0t = st.tile([128, 8], F32)

        pdps = pp.tile([NB, NA], F32, tag="pdps")   # 3 PSUM banks
        m2ps = pp.tile([128, K + 2], F32, tag="m2")
        tps = pp.tile([NB, 128], F32, tag="m2")     # preamble only; share bank

        # ---------------- preamble: loads ----------------
        nc.sync.dma_start(
            out=dsb[:],
            in_=d_mat[:, :].rearrange("(i p) a -> p i a", p=128),
        )
        nc.sync.dma_start(
            out=ysb[:],
            in_=y_t[:, :].rearrange("(i p) b -> p i b", p=128),
        )
        nc.sync.dma_start(out=rhs_t[:], in_=rhs_init[:, :, :])
        nc.sync.dma_start(out=rhsi[:], in_=rhs_init[:, :, :])
        nc.sync.dma_start(out=bdmask[:], in_=bdmask_in[:, :, :, :])
        nc.sync.dma_start(out=wmask[:], in_=wmask_in[:, :, :, :])
        nc.sync.dma_start(out=negsel[:], in_=negsel_in[:, :, :])
        nc.sync.dma_start(out=betabase[:], in_=betabase_in[:, :])
        nc.sync.dma_start(out=xbase[:], in_=xbase_in[:, :])
        nc.sync.dma_start(out=gselx[:], in_=gselx_in[:, :])
        nc.sync.dma_start(out=backsel[:], in_=backsel_in[:, :])
        nc.sync.dma_start(out=jmask[:], in_=jmask_in[:, :, :])
        nc.sync.dma_start(out=bsel16[:], in_=bsel16_in[:, :])
        nc.sync.dma_start(out=rjmask[:], in_=rjmask_in[:, :, :])
        make_identity(nc, id128[:])

        nc.vector.memset(fillr[:], 0.0)
        nc.vector.memset(vmat[:], 0.0)
        nc.vector.memset(nzw[:], 0.0)
        nc.vector.memset(offs[:], 0.0)
        nc.vector.memset(gsmP[:], 0.0)

        def warm_fill(dep_ap):
            """Dummy bf16 matmul: keeps the PE HAM activity window busy so
            fp32 matmuls run at 2.4 GHz. Output is never read; the lhsT
            bitcast ties it to per-phase state so the scheduler spreads
            the fillers across the timeline."""
            fps = pp.tile([2, 512], F32, tag="mx")
            nc.tensor.matmul(
                fps[:],
                lhsT=dep_ap.bitcast(mybir.dt.bfloat16)[:, 0:2],
                rhs=fillr[:],
                start=True,
                stop=True,
            )

        # ---------------- G = D^T D  and  proj0 ----------------
        with tc.tile_pool(name="gp", bufs=2, space="PSUM") as gp:
            pass
            for m in range(9):
                mw = 128 if m < 8 else 1
                msl = slice(m * 128, m * 128 + mw)
                for (n0, nl) in NCHUNKS:
                    gps = gp.tile([128, 512], F32, tag="gps")
                    for kk in range(8):
                        nc.tensor.matmul(
                            gps[:mw, :nl],
                            lhsT=dsb[:, kk, msl],
                            rhs=dsb[:, kk, n0 : n0 + nl],
                            start=(kk == 0),
                            stop=(kk == 7),
                        )
                    nc.scalar.copy(
                        out=gstage[:mw, m, n0 : n0 + nl], in_=gps[:mw, :nl]
                    )
                # proj0^T chunk rides along on the same lhsT
                pps = pp.tile([128, NB], F32, tag="pps")
                for kk in range(8):
                    nc.tensor.matmul(
                        pps[:mw, :],
                        lhsT=dsb[:, kk, msl],
                        rhs=ysb[:, kk, :],
                        start=(kk == 0),
                        stop=(kk == 7),
                    )
                # transpose proj0^T chunk into batch-major inject rows of bigt
                nc.vector.tensor_copy(p0t[:mw, :], pps[:mw, :])
                nc.tensor.transpose(tps[:, :mw], p0t[:mw, :], id128[:mw, :mw])
                nc.scalar.copy(out=bigt[0:NB, 0, msl], in_=tps[:, :mw])
                warm_fill(p0t[0:NB, 0:1])

        # G -> DRAM (rows 0..1023 from m<8, row 1024 separately)
        nc.sync.dma_start(
            out=grows_ap[0:1024, :].rearrange("(m p) c -> p m c", p=128),
            in_=gstage[:, 0:8, :],
        )
        nc.sync.dma_start(out=grows_ap[1024:1025, :], in_=gstage[0:1, 8, :])
        # proj0 (batch-major) -> DRAM rows 1025..1032
        nc.sync.dma_start(out=grows_ap[NA : NA + NB, :], in_=bigt[0:NB, 0, :])

        # ---------------- OMP iterations ----------------
        for i in range(K):
            rows = 8 + 8 * i
            nk = _cdiv(rows, 128)
            for ck in range(nk):
                cnt = min(128, rows - 128 * ck)
                for (n0, nl) in NCHUNKS:
                    nc.tensor.matmul(
                        pdps[:, n0 : n0 + nl],
                        lhsT=rhs_t[0:cnt, ck, :],
                        rhs=bigt[0:cnt, ck, n0 : n0 + nl],
                        start=(ck == 0),
                        stop=(ck == nk - 1),
                    )
            nc.scalar.activation(pdabs[:], pdps[:], AF.Abs)
            nc.vector.max(out=mx8[:], in_=pdabs[:])
            nc.vector.max_index(mi8[:], mx8[:], pdabs[:])
            warm_fill(pdabs[:, 0:1])
            warm_fill(mx8[:, 0:1])
            nc.vector.tensor_copy(idxf[:], mi8[:, 0:1])
            warm_fill(idxf[:, 0:1])
            nc.vector.tensor_copy(idxm[:, i : i + 1], idxf[:])

            # gather offsets: cols [0:i]=g, [i]=diag, [i+1]=beta
            if i > 0:
                nc.vector.scalar_tensor_tensor(
                    out=offs[:, 0:i],
                    in0=idxf[:].to_broadcast([NB, i]),
                    scalar=float(NA),
                    in1=idxm[:, 0:i],
                    op0=OP.mult,
                    op1=OP.add,
                )
            nc.vector.tensor_scalar_mul(offs[:, i : i + 1], idxf[:], float(NA + 1))
            nc.vector.tensor_scalar(
                out=offs[:, i + 1 : i + 2],
                in0=idxf[:],
                scalar1=betabase[:],
                scalar2=None,
                op0=OP.add,
            )
            # hw indirect DMA gathers one index per destination partition:
            # expand offs [8, j] -> partition-major rows r = 8j + b via matmul,
            # gather one element per partition, then collapse back to [8, j].
            nitem = 8 * (i + 2)
            nkg = _cdiv(nitem, 128)
            gsmps = pp.tile([NB, K + 2], F32, tag="mx")
            nc.tensor.matmul(
                m2ps[:], lhsT=gselx[:], rhs=offs[:], start=True, stop=True
            )
            for ck in range(nkg):
                cntg = min(128, nitem - 128 * ck)
                nc.vector.tensor_tensor(
                    out=prodg[:], in0=m2ps[:], in1=jmask[:, ck, :], op=OP.mult
                )
                nc.vector.tensor_reduce(
                    out=offsP[:, ck : ck + 1], in_=prodg[:], axis=AX.X, op=OP.add
                )
                nc.vector.tensor_copy(
                    offsPu[:, ck : ck + 1], offsP[:, ck : ck + 1]
                )
                nc.gpsimd.indirect_dma_start(
                    out=gsmP[0:cntg, ck : ck + 1],
                    out_offset=None,
                    in_=gflat_ap,
                    in_offset=bass.IndirectOffsetOnAxis(
                        ap=offsPu[0:cntg, ck : ck + 1], axis=0
                    ),
                )
                warm_fill(offsP[0:NB, ck : ck + 1])
                nc.vector.tensor_scalar(
                    out=grhs[:, ck, :], in0=jmask[:, ck, :],
                    scalar1=gsmP[:, ck : ck + 1], scalar2=None, op0=OP.mult,
                )
                nc.tensor.matmul(
                    gsmps[:], lhsT=backsel[:], rhs=grhs[:, ck, :],
                    start=(ck == 0), stop=(ck == nkg - 1),
                )
            nc.vector.tensor_copy(gsm[:], gsmps[:])
            warm_fill(gsm[:, 0:1])
            if i < K - 1:
                r0 = 8 + 8 * i
                gck, gp0 = r0 // 128, r0 % 128
                nc.gpsimd.indirect_dma_start(
                    out=bigt[gp0 : gp0 + 8, gck, :],
                    out_offset=None,
                    in_=grows_ap,
                    in_offset=bass.IndirectOffsetOnAxis(ap=mi8[:, 0:1], axis=0),
                )

            d_ap = gsm[:, i : i + 1]
            b_ap = gsm[:, i + 1 : i + 2]
            if i == 0:
                nc.vector.tensor_scalar_add(s_t[:], d_ap, REG)
                nc.vector.reciprocal(sinv_v[:, 0:1], s_t[:])
                nc.vector.tensor_copy(bvec[:, 0:1], b_ap)
                nc.vector.scalar_tensor_tensor(
                    out=alpha[:], in0=b_ap, scalar=-1.0,
                    in1=sinv_v[:, 0:1], op0=OP.mult, op1=OP.mult,
                )
                nc.vector.tensor_scalar_mul(nzw[:, 0:1], alpha[:], -1.0)
                nc.vector.memset(vmat[:, 0:1, 0:1], -1.0)
            else:
                g_ap = gsm[:, 0:i]
                nc.vector.tensor_tensor(
                    out=tmp3[:, 0:i, 0:i],
                    in0=vmat[:, 0:i, 0:i],
                    in1=g_ap.unsqueeze(1).to_broadcast([NB, i, i]),
                    op=OP.mult,
                )
                nc.vector.tensor_reduce(
                    out=c_t[:, 0:i], in_=tmp3[:, 0:i, 0:i], axis=AX.X, op=OP.add
                )
                nc.vector.tensor_tensor(
                    out=ct_t[:, 0:i], in0=c_t[:, 0:i], in1=sinv_v[:, 0:i], op=OP.mult
                )
                nc.vector.tensor_tensor(
                    out=tmp4[:, 0:i, 0:i],
                    in0=vmat[:, 0:i, 0:i].transpose([0, 2, 1]),
                    in1=ct_t[:, 0:i].unsqueeze(1).to_broadcast([NB, i, i]),
                    op=OP.mult,
                )
                nc.vector.tensor_reduce(
                    out=u_t[:, 0:i], in_=tmp4[:, 0:i, 0:i], axis=AX.X, op=OP.add
                )
                warm_fill(u_t[:, 0:1])
                nc.vector.tensor_tensor(
                    out=tmp5[:, 0:i], in0=g_ap, in1=u_t[:, 0:i], op=OP.mult
                )
                nc.vector.tensor_reduce(
                    out=sdot[:], in_=tmp5[:, 0:i], axis=AX.X, op=OP.add
                )
                nc.vector.scalar_tensor_tensor(
                    out=s_t[:], in0=d_ap, scalar=REG, in1=sdot[:],
                    op0=OP.add, op1=OP.subtract,
                )
                nc.vector.reciprocal(sinv_v[:, i : i + 1], s_t[:])
                nc.vector.tensor_tensor(
                    out=tmp5[:, 0:i], in0=u_t[:, 0:i], in1=bvec[:, 0:i], op=OP.mult
                )
                nc.vector.tensor_reduce(
                    out=ubdot[:], in_=tmp5[:, 0:i], axis=AX.X, op=OP.add
                )
                nc.vector.scalar_tensor_tensor(
                    out=alpha[:], in0=ubdot[:], scalar=b_ap,
                    in1=sinv_v[:, i : i + 1], op0=OP.subtract, op1=OP.mult,
                )
                nc.vector.scalar_tensor_tensor(
                    out=nzw[:, 0:i], in0=u_t[:, 0:i], scalar=alpha[:],
                    in1=nzw[:, 0:i], op0=OP.mult, op1=OP.add,
                )
                nc.vector.tensor_scalar_mul(nzw[:, i : i + 1], alpha[:], -1.0)
                nc.vector.tensor_copy(vmat[:, i, 0:i], u_t[:, 0:i])
                nc.vector.memset(vmat[:, i : i + 1, i : i + 1], -1.0)
                nc.vector.tensor_copy(bvec[:, i : i + 1], b_ap)
                warm_fill(alpha[:, 0:1])

            # rebuild matmul weights (rows 8..8+8(i+1)) for next iteration
            if i < K - 1:
                rows_next = 8 + 8 * (i + 1)
                nk2 = _cdiv(rows_next, 128)
                # M1 expansion differs per chunk only through bdmask; one
                # matmul + one fused tt/reduce/add across all live chunks.
                m1ps = pp.tile([128, K], F32, tag="mx")
                nc.tensor.matmul(
                    m1ps[:], lhsT=negsel[:, 0, :], rhs=nzw[:],
                    start=True, stop=True,
                )
                nc.vector.tensor_tensor(
                    out=prod2_t[:, 0:nk2, :, :],
                    in0=m1ps[:]
                    .unsqueeze(1)
                    .unsqueeze(1)
                    .to_broadcast([128, nk2, NB, K]),
                    in1=bdmask[:, 0:nk2, :, :],
                    op=OP.mult,
                )
                nc.vector.tensor_reduce(
                    out=rhsr2[:, 0:nk2, :], in_=prod2_t[:, 0:nk2, :, :],
                    axis=AX.X, op=OP.add,
                )
                nc.vector.tensor_tensor(
                    out=rhs_t[:, 0:nk2, :], in0=rhsr2[:, 0:nk2, :],
                    in1=rhsi[:, 0:nk2, :], op=OP.add,
                )
                warm_fill(rhsr2[0:NB, 0, 0:1])

        # ---------------- reconstruction ----------------
        # row r = 128*ck + p of the gather maps to (b = p//16, k = 16ck + p%16)
        nc.vector.tensor_scalar(
            out=xoff[:], in0=idxm[:], scalar1=xbase[:], scalar2=None, op0=OP.add
        )
        xt_rows = xt_pad[:, :]
        nc.tensor.matmul(
            m2ps[:, 0:K], lhsT=bsel16[:], rhs=xoff[:], start=True, stop=True
        )
        for ck in range(2):
            m1ps = pp.tile([128, K], F32, tag="mx")
            nc.vector.tensor_tensor(
                out=prodr[:], in0=m2ps[:, 0:K], in1=rjmask[:, ck, :], op=OP.mult
            )
            nc.vector.tensor_reduce(
                out=xoffP[:, ck : ck + 1], in_=prodr[:], axis=AX.X, op=OP.add
            )
            nc.vector.tensor_copy(xoffPu[:, ck : ck + 1], xoffP[:, ck : ck + 1])
            nc.gpsimd.indirect_dma_start(
                out=xsel[:, ck, :],
                out_offset=None,
                in_=xt_rows,
                in_offset=bass.IndirectOffsetOnAxis(
                    ap=xoffPu[:, ck : ck + 1], axis=0
                ),
            )
            nc.tensor.matmul(
                m1ps[:], lhsT=bsel16[:], rhs=nzw[:], start=True, stop=True
            )
            nc.vector.tensor_tensor(
                out=prod_t[:, :, :],
                in0=m1ps[:, 0:K].unsqueeze(1).to_broadcast([128, NB, K]),
                in1=wmask[:, ck, :, :],
                op=OP.mult,
            )
            nc.vector.tensor_reduce(
                out=wsel[:, ck, :], in_=prod_t[:, :, :], axis=AX.X, op=OP.add
            )
        ops = pp.tile([NB, L], F32, tag="pdps")
        for ck in range(2):
            for (n0, nl) in [(0, 512), (512, 512)]:
                nc.tensor.matmul(
                    ops[:, n0 : n0 + nl],
                    lhsT=wsel[:, ck, :],
                    rhs=xsel[:, ck, n0 : n0 + nl],
                    start=(ck == 0),
                    stop=(ck == 1),
                )
        nc.scalar.copy(out=outsb[:], in_=ops[:])
        nc.sync.dma_start(out=out_r[:, :], in_=outsb[:])

    return nc


_NC_CACHE = None


def _get_program():
    global _NC_CACHE
    if _NC_CACHE is None:
        _NC_CACHE = _build_program()
    return _NC_CACHE


def _host_constants():
    c = {}
    rhs_init = np.zeros((128, 3, NB), np.float32)
    for b in range(NB):
        rhs_init[b, 0, b] = 1.0
    bdmask = np.zeros((128, 3, NB, K), np.float32)
    negsel = np.zeros((NB, 3, 128), np.float32)
    for ck in range(3):
        for p in range(128):
            negsel[p % 8, ck, p] = -1.0     # validity filtering lives in bdmask
            r = ck * 128 + p
            if r < 8 or r >= 8 + 8 * K:
                continue
            b, kk = (r - 8) % 8, (r - 8) // 8
            bdmask[p, ck, b, kk] = 1.0
    wmask = np.zeros((128, 2, NB, K), np.float32)
    bsel16 = np.zeros((NB, 128), np.float32)
    rjmask = np.zeros((128, 2, K), np.float32)
    for p in range(128):
        bsel16[p // 16, p] = 1.0
        for ck in range(2):
            b, kk = p // 16, ck * 16 + p % 16
            wmask[p, ck, b, kk] = 1.0
            rjmask[p, ck, kk] = 1.0
    gselx = np.zeros((NB, 128), np.float32)
    backsel = np.zeros((128, NB), np.float32)
    jmask = np.zeros((128, 3, K + 2), np.float32)
    for p in range(128):
        gselx[p % 8, p] = 1.0
        backsel[p, p % 8] = 1.0
        for ck in range(3):
            j = 16 * ck + p // 8
            if j < K + 2:
                jmask[p, ck, j] = 1.0
    betabase = (NA * NA + np.arange(NB, dtype=np.float32)[:, None] * NA).astype(
        np.float32
    )
    xbase = (np.arange(NB, dtype=np.float32)[:, None] * NA).astype(np.float32)
    c.update(
        rhs_init=rhs_init, bdmask_in=bdmask, negsel_in=negsel, wmask_in=wmask,
        betabase_in=betabase, xbase_in=xbase, gselx_in=gselx, backsel_in=backsel,
        jmask_in=jmask, bsel16_in=bsel16, rjmask_in=rjmask,
    )
    return c


def kernel(X, y):
    X = np.ascontiguousarray(np.asarray(X, dtype=np.float32))
    y = np.ascontiguousarray(np.asarray(y, dtype=np.float32))
    B = X.shape[0]
    assert B == NCORES * NB and X.shape[1:] == (L, L) and y.shape == (B, L, 1)

    nc = _get_program()
    consts = _host_constants()

    d_mat = np.ascontiguousarray(
        np.concatenate([X[0], np.ones((L, 1), np.float32)], axis=1)
    )

    in_maps = []
    for c in range(NCORES):
        sl = slice(c * NB, (c + 1) * NB)
        y_t = np.ascontiguousarray(y[sl, :, 0].T)
        xt = np.ascontiguousarray(X[sl].transpose(0, 2, 1))          # [NB, A, L]
        xt_pad = np.concatenate(
            [xt, np.ones((NB, 1, L), np.float32)], axis=1
        ).reshape(NB * NA, L)
        m = {"d_mat": d_mat, "y_t": y_t, "xt_pad": np.ascontiguousarray(xt_pad)}
        m.update(consts)
        in_maps.append(m)

    res = run_bass_kernel_spmd(nc, in_maps, core_ids=list(range(NCORES)))
    out = np.concatenate([res.results[c]["out_r"] for c in range(NCORES)], axis=0)
    return out.reshape(B, L, 1).astype(np.float32)


def profile_once(X, y):
    """Run once with NTFF tracing; returns exec_time_ns (max across cores)."""
    X = np.ascontiguousarray(np.asarray(X, dtype=np.float32))
    y = np.ascontiguousarray(np.asarray(y, dtype=np.float32))
    nc = _get_program()
    consts = _host_constants()
    d_mat = np.ascontiguousarray(
        np.concatenate([X[0], np.ones((L, 1), np.float32)], axis=1)
    )
    in_maps = []
    for c in range(NCORES):
        sl = slice(c * NB, (c + 1) * NB)
        y_t = np.ascontiguousarray(y[sl, :, 0].T)
        xt = np.ascontiguousarray(X[sl].transpose(0, 2, 1))
        xt_pad = np.concatenate(
            [xt, np.ones((NB, 1, L), np.float32)], axis=1
        ).reshape(NB * NA, L)
        m = {"d_mat": d_mat, "y_t": y_t, "xt_pad": np.ascontiguousarray(xt_pad)}
        m.update(consts)
        in_maps.append(m)
    res = run_bass_kernel_spmd(
        nc, in_maps, core_ids=list(range(NCORES)), trace=True
    )
    return res.exec_time_ns


# revision 31
# speedup vs baseline: 1.0062x; 1.0062x over previous
"""Differentiable OMP (top-k masking) Trainium2 kernel.

Strategy (pure data parallelism over batch, 8 batches/core on 8 cores):
  The straight-through softmax terms cancel numerically in the forward pass,
  so each OMP iteration reduces to:
    pd    = proj0 - nzW @ G[idx_sel, :]        (argmax drive)
    idx_i = argmax |pd|
    solve (G[S,S] + reg I) nzW = proj0[S] incrementally (bordered inverse,
    rank-one product form) -- all O(i^2) work batched on 8 partitions.
  where G = D^T D (Gram of the shared dictionary) and proj0 = y @ D are
  computed once on device.  The final reconstruction gathers the 32 selected
  dictionary columns per batch from the X shard and combines with nzW on the
  tensor engine.  Only ~5 MB of the 32 MB X shard is ever read (indirect
  DMA gather with on-device indices).
"""

import os
import sys

for _p in ("/opt/trn_rl_repo", "/root/.axon_site/_ro/trn_rl_repo"):
    if os.path.isdir(_p) and _p not in sys.path:
        sys.path.insert(0, _p)

import numpy as np

import concourse.bass as bass
import concourse.mybir as mybir
import concourse.tile as tile
from concourse.bass_utils import run_bass_kernel_spmd
from concourse.masks import make_identity
from concourse.vector_clock import ScopedClock

F32 = mybir.dt.float32
U32 = mybir.dt.uint32
OP = mybir.AluOpType
AF = mybir.ActivationFunctionType
AX = mybir.AxisListType

NCORES = 8
NB = 8            # batches per core
L = 1024          # signal length
NA = 1025         # atoms (1024 + bias column)
K = 32            # n_nonzero_coefs
REG = float(np.log1p(np.exp(np.float32(-5.0), dtype=np.float32), dtype=np.float32))
GBUF_ROWS = NA + NB          # G rows then proj0 rows
NCHUNKS = [(0, 512), (512, 512), (1024, 1)]


_PATCHED = False


def _patch_tile_drain():
    """This walrus build rejects >1 sync waits per instruction: split the
    final-drain waits onto SP nops, and split any lowered instruction's
    extra waits onto same-engine nops."""
    global _PATCHED
    if _PATCHED:
        return
    _PATCHED = True

    _orig_commit_and_lower = tile.TileContext._commit_and_lower

    def _commit_and_lower_split(self, inst, original_block, old_bb_map, bb_to_exit):
        si = getattr(inst, "sync_info", None)
        if si is not None and si.on_wait and len(si.on_wait) > 1:
            waits = list(si.on_wait)
            for j, w in enumerate(waits[1:]):
                nop = mybir.InstNoOp(
                    name=f"{inst.name}-wsplit{j}", ins=[], outs=[], engine=inst.engine
                )
                nop.sync_info = mybir.SyncInfo(on_wait=[w], on_update=[])
                _orig_commit_and_lower(self, nop, original_block, old_bb_map, bb_to_exit)
            inst.sync_info = mybir.SyncInfo(
                on_wait=[waits[0]],
                on_update=list(si.on_update) if si.on_update else [],
            )
        return _orig_commit_and_lower(self, inst, original_block, old_bb_map, bb_to_exit)

    def _drain_and_barrier_split(self, tick_clock, wait_clock):
        nc = self.nc
        drain_inst = nc.sync.drain()
        wait_clock.add_sem_waits(
            drain_inst.ins, ScopedClock({None: tick_clock.global_clock})
        )
        si = drain_inst.ins.sync_info
        waits = list(si.on_wait) if si is not None and si.on_wait else []
        if len(waits) > 1:
            drain_inst.ins.sync_info = mybir.SyncInfo(
                on_wait=[waits[0]],
                on_update=list(si.on_update) if si.on_update else [],
            )
            for w in waits[1:]:
                n = nc.sync.nop()
                n.ins.sync_info = mybir.SyncInfo(on_wait=[w], on_update=[])

        nc.all_engine_barrier()
        assert self.sems is not None
        popped = nc._tile_sem_poison_stack.pop()
        assert popped is self._sem_poison
        nc.clear_and_free_semaphores(list(self.sems.allocated().values()))
        nc.all_engine_barrier()

    tile.TileContext._drain_and_barrier = _drain_and_barrier_split
    tile.TileContext._commit_and_lower = _commit_and_lower_split


def _cdiv(a, b):
    return (a + b - 1) // b


def _build_program():
    _patch_tile_drain()
    nc = bass.Bass()

    d_mat = nc.dram_tensor("d_mat", [L, NA], F32, kind="ExternalInput")
    y_t = nc.dram_tensor("y_t", [L, NB], F32, kind="ExternalInput")
    xt_pad = nc.dram_tensor("xt_pad", [NB * NA, L], F32, kind="ExternalInput")
    rhs_init = nc.dram_tensor("rhs_init", [128, 3, NB], F32, kind="ExternalInput")
    bdmask_in = nc.dram_tensor("bdmask_in", [128, 3, NB, K], F32, kind="ExternalInput")
    negsel_in = nc.dram_tensor("negsel_in", [NB, 3, 128], F32, kind="ExternalInput")
    wmask_in = nc.dram_tensor("wmask_in", [128, 2, NB, K], F32, kind="ExternalInput")
    betabase_in = nc.dram_tensor("betabase_in", [NB, 1], F32, kind="ExternalInput")
    xbase_in = nc.dram_tensor("xbase_in", [NB, 1], F32, kind="ExternalInput")
    # partition-expansion helpers (indirect DMA wants one index per partition)
    gselx_in = nc.dram_tensor("gselx_in", [NB, 128], F32, kind="ExternalInput")
    backsel_in = nc.dram_tensor("backsel_in", [128, NB], F32, kind="ExternalInput")
    jmask_in = nc.dram_tensor("jmask_in", [128, 3, K + 2], F32, kind="ExternalInput")
    bsel16_in = nc.dram_tensor("bsel16_in", [NB, 128], F32, kind="ExternalInput")
    rjmask_in = nc.dram_tensor("rjmask_in", [128, 2, K], F32, kind="ExternalInput")
    out_r = nc.dram_tensor("out_r", [NB, L], F32, kind="ExternalOutput")
    gbuf = nc.dram_tensor("gbuf", [GBUF_ROWS * NA, 1], F32, kind="Internal")

    gflat_ap = gbuf[:, :]                                            # element gather
    grows_ap = gbuf[:, :].rearrange("(r c) x -> r (c x)", c=NA)      # row gather

    import contextlib

    with tile.TileContext(nc) as tc, contextlib.ExitStack() as ctx:
        st = ctx.enter_context(tc.tile_pool(name="st", bufs=1))
        pp = ctx.enter_context(tc.tile_pool(name="pp", bufs=1, space="PSUM"))

        # ---------------- persistent state ----------------
        dsb = st.tile([128, 8, NA], F32)          # D, L split in 8 chunks
        ysb = st.tile([128, 8, NB], F32)          # y^T
        gstage = st.tile([128, 9, NA], F32)       # G staging (rows m*128+p)
        bigt = st.tile([128, 3, NA], F32)         # K-rows: proj0(8) + G rows
        rhs_t = st.tile([128, 3, NB], F32)        # matmul weights per K-row
        rhsi = st.tile([128, 3, NB], F32)         # inject-row pattern (ID8)
        rhsr = st.tile([128, NB], F32)            # rebuild scratch
        bdmask = st.tile([128, 3, NB, K], F32)
        wmask = st.tile([128, 2, NB, K], F32)
        negsel = st.tile([NB, 3, 128], F32)
        betabase = st.tile([NB, 1], F32)
        xbase = st.tile([NB, 1], F32)
        id128 = st.tile([128, 128], F32)
        gselx = st.tile([NB, 128], F32)
        backsel = st.tile([128, NB], F32)
        jmask = st.tile([128, 3, K + 2], F32)
        bsel16 = st.tile([NB, 128], F32)
        rjmask = st.tile([128, 2, K], F32)
        prodg = st.tile([128, K + 2], F32)
        fillr = st.tile([NB, 512], mybir.dt.bfloat16)   # HAM warm-keeper rhs
        prod2_t = st.tile([128, 3, NB, K], F32)
        rhsr2 = st.tile([128, 3, NB], F32)
        offsP = st.tile([128, 3], F32)
        offsPu = st.tile([128, 3], U32)
        gsmP = st.tile([128, 3], F32)
        grhs = st.tile([128, 3, K + 2], F32)
        xoffP = st.tile([128, 2], F32)
        xoffPu = st.tile([128, 2], U32)
        prodr = st.tile([128, K], F32)

        vmat = st.tile([NB, K, K], F32)
        nzw = st.tile([NB, K], F32)
        sinv_v = st.tile([NB, K], F32)
        bvec = st.tile([NB, K], F32)
        idxm = st.tile([NB, K], F32)
        gsm = st.tile([NB, K + 2], F32)
        offs = st.tile([NB, K + 2], F32)
        idxf = st.tile([NB, 1], F32)
        u_t = st.tile([NB, K], F32)
        c_t = st.tile([NB, K], F32)
        ct_t = st.tile([NB, K], F32)
        tmp3 = st.tile([NB, K, K], F32)
        tmp4 = st.tile([NB, K, K], F32)
        tmp5 = st.tile([NB, K], F32)
        sdot = st.tile([NB, 1], F32)
        s_t = st.tile([NB, 1], F32)
        alpha = st.tile([NB, 1], F32)
        ubdot = st.tile([NB, 1], F32)
        pdabs = st.tile([NB, NA], F32)
        mx8 = st.tile([NB, 8], F32)
        mi8 = st.tile([NB, 8], U32)
        prod_t = st.tile([128, NB, K], F32)
        xoff = st.tile([NB, K], F32)
        xsel = st.tile([128, 2, L], F32)
        wsel = st.tile([128, 2, NB], F32)
        outsb = st.tile([NB, L], F32)
        p0t = st.tile([128, 8], F32)

        pdps = pp.tile([NB, NA], F32, tag="pdps")   # 3 PSUM banks
        m2ps = pp.tile([128, K + 2], F32, tag="m2")
        tps = pp.tile([NB, 128], F32, tag="m2")     # preamble only; share bank

        # ---------------- preamble: loads ----------------
        nc.sync.dma_start(
            out=dsb[:],
            in_=d_mat[:, :].rearrange("(i p) a -> p i a", p=128),
        )
        nc.sync.dma_start(
            out=ysb[:],
            in_=y_t[:, :].rearrange("(i p) b -> p i b", p=128),
        )
        nc.sync.dma_start(out=rhs_t[:], in_=rhs_init[:, :, :])
        nc.sync.dma_start(out=rhsi[:], in_=rhs_init[:, :, :])
        nc.sync.dma_start(out=bdmask[:], in_=bdmask_in[:, :, :, :])
        nc.sync.dma_start(out=wmask[:], in_=wmask_in[:, :, :, :])
        nc.sync.dma_start(out=negsel[:], in_=negsel_in[:, :, :])
        nc.sync.dma_start(out=betabase[:], in_=betabase_in[:, :])
        nc.sync.dma_start(out=xbase[:], in_=xbase_in[:, :])
        nc.sync.dma_start(out=gselx[:], in_=gselx_in[:, :])
        nc.sync.dma_start(out=backsel[:], in_=backsel_in[:, :])
        nc.sync.dma_start(out=jmask[:], in_=jmask_in[:, :, :])
        nc.sync.dma_start(out=bsel16[:], in_=bsel16_in[:, :])
        nc.sync.dma_start(out=rjmask[:], in_=rjmask_in[:, :, :])
        make_identity(nc, id128[:])

        nc.vector.memset(fillr[:], 0.0)
        nc.vector.memset(vmat[:], 0.0)
        nc.vector.memset(nzw[:], 0.0)
        nc.vector.memset(offs[:], 0.0)
        nc.vector.memset(gsmP[:], 0.0)

        _fill_pool = [pp]

        def warm_fill(dep_ap, tag="mx"):
            """Dummy bf16 matmul: keeps the PE HAM activity window busy so
            fp32 matmuls run at 2.4 GHz. Output is never read; the lhsT
            bitcast ties it to per-phase state so the scheduler spreads
            the fillers across the timeline."""
            fps = _fill_pool[0].tile([2, 512], F32, tag=tag)
            nc.tensor.matmul(
                fps[:],
                lhsT=dep_ap.bitcast(mybir.dt.bfloat16)[:, 0:2],
                rhs=fillr[:],
                start=True,
                stop=True,
            )

        # ---------------- G = D^T D  and  proj0 ----------------
        with tc.tile_pool(name="gp", bufs=2, space="PSUM") as gp:
            pass
            for m in range(9):
                mw = 128 if m < 8 else 1
                msl = slice(m * 128, m * 128 + mw)
                for (n0, nl) in NCHUNKS:
                    gps = gp.tile([128, 512], F32, tag="gps")
                    for kk in range(8):
                        nc.tensor.matmul(
                            gps[:mw, :nl],
                            lhsT=dsb[:, kk, msl],
                            rhs=dsb[:, kk, n0 : n0 + nl],
                            start=(kk == 0),
                            stop=(kk == 7),
                        )
                    nc.scalar.copy(
                        out=gstage[:mw, m, n0 : n0 + nl], in_=gps[:mw, :nl]
                    )
                # proj0^T chunk rides along on the same lhsT
                pps = pp.tile([128, NB], F32, tag="mx")
                for kk in range(8):
                    nc.tensor.matmul(
                        pps[:mw, :],
                        lhsT=dsb[:, kk, msl],
                        rhs=ysb[:, kk, :],
                        start=(kk == 0),
                        stop=(kk == 7),
                    )
                # transpose proj0^T chunk into batch-major inject rows of bigt
                nc.vector.tensor_copy(p0t[:mw, :], pps[:mw, :])
                nc.tensor.transpose(tps[:, :mw], p0t[:mw, :], id128[:mw, :mw])
                nc.scalar.copy(out=bigt[0:NB, 0, msl], in_=tps[:, :mw])
                warm_fill(p0t[0:NB, 0:1], tag="gps")

        fl = ctx.enter_context(tc.tile_pool(name="fl", bufs=1, space="PSUM"))
        _fill_pool[0] = fl

        # G -> DRAM (rows 0..1023 from m<8, row 1024 separately)
        nc.sync.dma_start(
            out=grows_ap[0:1024, :].rearrange("(m p) c -> p m c", p=128),
            in_=gstage[:, 0:8, :],
        )
        nc.sync.dma_start(out=grows_ap[1024:1025, :], in_=gstage[0:1, 8, :])
        # proj0 (batch-major) -> DRAM rows 1025..1032
        nc.sync.dma_start(out=grows_ap[NA : NA + NB, :], in_=bigt[0:NB, 0, :])

        # ---------------- OMP iterations ----------------
        for i in range(K):
            rows = 8 + 8 * i
            nk = _cdiv(rows, 128)
            for ck in range(nk):
                cnt = min(128, rows - 128 * ck)
                for (n0, nl) in NCHUNKS:
                    nc.tensor.matmul(
                        pdps[:, n0 : n0 + nl],
                        lhsT=rhs_t[0:cnt, ck, :],
                        rhs=bigt[0:cnt, ck, n0 : n0 + nl],
                        start=(ck == 0),
                        stop=(ck == nk - 1),
                    )
            nc.scalar.activation(pdabs[:], pdps[:], AF.Abs)
            nc.vector.max(out=mx8[:], in_=pdabs[:])
            nc.vector.max_index(mi8[:], mx8[:], pdabs[:])
            warm_fill(pdabs[:, 0:1])
            warm_fill(mx8[:, 0:1])
            nc.vector.tensor_copy(idxf[:], mi8[:, 0:1])
            warm_fill(idxf[:, 0:1])
            nc.vector.tensor_copy(idxm[:, i : i + 1], idxf[:])

            # gather offsets: cols [0:i]=g, [i]=diag, [i+1]=beta
            if i > 0:
                nc.vector.scalar_tensor_tensor(
                    out=offs[:, 0:i],
                    in0=idxf[:].to_broadcast([NB, i]),
                    scalar=float(NA),
                    in1=idxm[:, 0:i],
                    op0=OP.mult,
                    op1=OP.add,
                )
            nc.vector.tensor_scalar_mul(offs[:, i : i + 1], idxf[:], float(NA + 1))
            nc.vector.tensor_scalar(
                out=offs[:, i + 1 : i + 2],
                in0=idxf[:],
                scalar1=betabase[:],
                scalar2=None,
                op0=OP.add,
            )
            # hw indirect DMA gathers one index per destination partition:
            # expand offs [8, j] -> partition-major rows r = 8j + b via matmul,
            # gather one element per partition, then collapse back to [8, j].
            nitem = 8 * (i + 2)
            nkg = _cdiv(nitem, 128)
            gsmps = pp.tile([NB, K + 2], F32, tag="mx")
            nc.tensor.matmul(
                m2ps[:], lhsT=gselx[:], rhs=offs[:], start=True, stop=True
            )
            for ck in range(nkg):
                cntg = min(128, nitem - 128 * ck)
                nc.vector.tensor_tensor(
                    out=prodg[:], in0=m2ps[:], in1=jmask[:, ck, :], op=OP.mult
                )
                nc.vector.tensor_reduce(
                    out=offsP[:, ck : ck + 1], in_=prodg[:], axis=AX.X, op=OP.add
                )
                nc.vector.tensor_copy(
                    offsPu[:, ck : ck + 1], offsP[:, ck : ck + 1]
                )
                nc.gpsimd.indirect_dma_start(
                    out=gsmP[0:cntg, ck : ck + 1],
                    out_offset=None,
                    in_=gflat_ap,
                    in_offset=bass.IndirectOffsetOnAxis(
                        ap=offsPu[0:cntg, ck : ck + 1], axis=0
                    ),
                )
                warm_fill(offsP[0:NB, ck : ck + 1])
                nc.vector.tensor_scalar(
                    out=grhs[:, ck, :], in0=jmask[:, ck, :],
                    scalar1=gsmP[:, ck : ck + 1], scalar2=None, op0=OP.mult,
                )
                nc.tensor.matmul(
                    gsmps[:], lhsT=backsel[:], rhs=grhs[:, ck, :],
                    start=(ck == 0), stop=(ck == nkg - 1),
                )
            nc.vector.tensor_copy(gsm[:], gsmps[:])
            warm_fill(gsm[:, 0:1])
            if i < K - 1:
                r0 = 8 + 8 * i
                gck, gp0 = r0 // 128, r0 % 128
                nc.gpsimd.indirect_dma_start(
                    out=bigt[gp0 : gp0 + 8, gck, :],
                    out_offset=None,
                    in_=grows_ap,
                    in_offset=bass.IndirectOffsetOnAxis(ap=mi8[:, 0:1], axis=0),
                )

            d_ap = gsm[:, i : i + 1]
            b_ap = gsm[:, i + 1 : i + 2]
            if i == 0:
                nc.vector.tensor_scalar_add(s_t[:], d_ap, REG)
                nc.vector.reciprocal(sinv_v[:, 0:1], s_t[:])
                nc.vector.tensor_copy(bvec[:, 0:1], b_ap)
                nc.vector.scalar_tensor_tensor(
                    out=alpha[:], in0=b_ap, scalar=-1.0,
                    in1=sinv_v[:, 0:1], op0=OP.mult, op1=OP.mult,
                )
                nc.vector.tensor_scalar_mul(nzw[:, 0:1], alpha[:], -1.0)
                nc.vector.memset(vmat[:, 0:1, 0:1], -1.0)
            else:
                g_ap = gsm[:, 0:i]
                nc.vector.tensor_tensor(
                    out=tmp3[:, 0:i, 0:i],
                    in0=vmat[:, 0:i, 0:i],
                    in1=g_ap.unsqueeze(1).to_broadcast([NB, i, i]),
                    op=OP.mult,
                )
                nc.vector.tensor_reduce(
                    out=c_t[:, 0:i], in_=tmp3[:, 0:i, 0:i], axis=AX.X, op=OP.add
                )
                nc.vector.tensor_tensor(
                    out=ct_t[:, 0:i], in0=c_t[:, 0:i], in1=sinv_v[:, 0:i], op=OP.mult
                )
                nc.vector.tensor_tensor(
                    out=tmp4[:, 0:i, 0:i],
                    in0=vmat[:, 0:i, 0:i].transpose([0, 2, 1]),
                    in1=ct_t[:, 0:i].unsqueeze(1).to_broadcast([NB, i, i]),
                    op=OP.mult,
                )
                nc.vector.tensor_reduce(
                    out=u_t[:, 0:i], in_=tmp4[:, 0:i, 0:i], axis=AX.X, op=OP.add
                )
                warm_fill(u_t[:, 0:1])
                nc.vector.tensor_tensor(
                    out=tmp5[:, 0:i], in0=g_ap, in1=u_t[:, 0:i], op=OP.mult
                )
                nc.vector.tensor_reduce(
                    out=sdot[:], in_=tmp5[:, 0:i], axis=AX.X, op=OP.add
                )
                nc.vector.scalar_tensor_tensor(
                    out=s_t[:], in0=d_ap, scalar=REG, in1=sdot[:],
                    op0=OP.add, op1=OP.subtract,
                )
                nc.vector.reciprocal(sinv_v[:, i : i + 1], s_t[:])
                nc.vector.tensor_tensor(
                    out=tmp5[:, 0:i], in0=u_t[:, 0:i], in1=bvec[:, 0:i], op=OP.mult
                )
                nc.vector.tensor_reduce(
                    out=ubdot[:], in_=tmp5[:, 0:i], axis=AX.X, op=OP.add
                )
                nc.vector.scalar_tensor_tensor(
                    out=alpha[:], in0=ubdot[:], scalar=b_ap,
                    in1=sinv_v[:, i : i + 1], op0=OP.subtract, op1=OP.mult,
                )
                nc.vector.scalar_tensor_tensor(
                    out=nzw[:, 0:i], in0=u_t[:, 0:i], scalar=alpha[:],
                    in1=nzw[:, 0:i], op0=OP.mult, op1=OP.add,
                )
                nc.vector.tensor_scalar_mul(nzw[:, i : i + 1], alpha[:], -1.0)
                nc.vector.tensor_copy(vmat[:, i, 0:i], u_t[:, 0:i])
                nc.vector.memset(vmat[:, i : i + 1, i : i + 1], -1.0)
                nc.vector.tensor_copy(bvec[:, i : i + 1], b_ap)
                warm_fill(alpha[:, 0:1])

            # rebuild matmul weights (rows 8..8+8(i+1)) for next iteration
            if i < K - 1:
                rows_next = 8 + 8 * (i + 1)
                nk2 = _cdiv(rows_next, 128)
                # M1 expansion differs per chunk only through bdmask; one
                # matmul + one fused tt/reduce/add across all live chunks.
                m1ps = pp.tile([128, K], F32, tag="mx")
                nc.tensor.matmul(
                    m1ps[:], lhsT=negsel[:, 0, :], rhs=nzw[:],
                    start=True, stop=True,
                )
                nc.vector.tensor_tensor(
                    out=prod2_t[:, 0:nk2, :, :],
                    in0=m1ps[:]
                    .unsqueeze(1)
                    .unsqueeze(1)
                    .to_broadcast([128, nk2, NB, K]),
                    in1=bdmask[:, 0:nk2, :, :],
                    op=OP.mult,
                )
                nc.vector.tensor_reduce(
                    out=rhsr2[:, 0:nk2, :], in_=prod2_t[:, 0:nk2, :, :],
                    axis=AX.X, op=OP.add,
                )
                nc.vector.tensor_tensor(
                    out=rhs_t[:, 0:nk2, :], in0=rhsr2[:, 0:nk2, :],
                    in1=rhsi[:, 0:nk2, :], op=OP.add,
                )
                warm_fill(rhsr2[0:NB, 0, 0:1])

        # ---------------- reconstruction ----------------
        # row r = 128*ck + p of the gather maps to (b = p//16, k = 16ck + p%16)
        nc.vector.tensor_scalar(
            out=xoff[:], in0=idxm[:], scalar1=xbase[:], scalar2=None, op0=OP.add
        )
        xt_rows = xt_pad[:, :]
        nc.tensor.matmul(
            m2ps[:, 0:K], lhsT=bsel16[:], rhs=xoff[:], start=True, stop=True
        )
        for ck in range(2):
            m1ps = pp.tile([128, K], F32, tag="mx")
            nc.vector.tensor_tensor(
                out=prodr[:], in0=m2ps[:, 0:K], in1=rjmask[:, ck, :], op=OP.mult
            )
            nc.vector.tensor_reduce(
                out=xoffP[:, ck : ck + 1], in_=prodr[:], axis=AX.X, op=OP.add
            )
            nc.vector.tensor_copy(xoffPu[:, ck : ck + 1], xoffP[:, ck : ck + 1])
            nc.gpsimd.indirect_dma_start(
                out=xsel[:, ck, :],
                out_offset=None,
                in_=xt_rows,
                in_offset=bass.IndirectOffsetOnAxis(
                    ap=xoffPu[:, ck : ck + 1], axis=0
                ),
            )
            nc.tensor.matmul(
                m1ps[:], lhsT=bsel16[:], rhs=nzw[:], start=True, stop=True
            )
            nc.vector.tensor_tensor(
                out=prod_t[:, :, :],
                in0=m1ps[:, 0:K].unsqueeze(1).to_broadcast([128, NB, K]),
                in1=wmask[:, ck, :, :],
                op=OP.mult,
            )
            nc.vector.tensor_reduce(
                out=wsel[:, ck, :], in_=prod_t[:, :, :], axis=AX.X, op=OP.add
            )
        ops = pp.tile([NB, L], F32, tag="pdps")
        for ck in range(2):
            for (n0, nl) in [(0, 512), (512, 512)]:
                nc.tensor.matmul(
                    ops[:, n0 : n0 + nl],
                    lhsT=wsel[:, ck, :],
                    rhs=xsel[:, ck, n0 : n0 + nl],
                    start=(ck == 0),
                    stop=(ck == 1),
                )
        nc.scalar.copy(out=outsb[:], in_=ops[:])
        nc.sync.dma_start(out=out_r[:, :], in_=outsb[:])

    return nc


_NC_CACHE = None


def _get_program():
    global _NC_CACHE
    if _NC_CACHE is None:
        _NC_CACHE = _build_program()
    return _NC_CACHE


def _host_constants():
    c = {}
    rhs_init = np.zeros((128, 3, NB), np.float32)
    for b in range(NB):
        rhs_init[b, 0, b] = 1.0
    bdmask = np.zeros((128, 3, NB, K), np.float32)
    negsel = np.zeros((NB, 3, 128), np.float32)
    for ck in range(3):
        for p in range(128):
            negsel[p % 8, ck, p] = -1.0     # validity filtering lives in bdmask
            r = ck * 128 + p
            if r < 8 or r >= 8 + 8 * K:
                continue
            b, kk = (r - 8) % 8, (r - 8) // 8
            bdmask[p, ck, b, kk] = 1.0
    wmask = np.zeros((128, 2, NB, K), np.float32)
    bsel16 = np.zeros((NB, 128), np.float32)
    rjmask = np.zeros((128, 2, K), np.float32)
    for p in range(128):
        bsel16[p // 16, p] = 1.0
        for ck in range(2):
            b, kk = p // 16, ck * 16 + p % 16
            wmask[p, ck, b, kk] = 1.0
            rjmask[p, ck, kk] = 1.0
    gselx = np.zeros((NB, 128), np.float32)
    backsel = np.zeros((128, NB), np.float32)
    jmask = np.zeros((128, 3, K + 2), np.float32)
    for p in range(128):
        gselx[p % 8, p] = 1.0
        backsel[p, p % 8] = 1.0
        for ck in range(3):
            j = 16 * ck + p // 8
            if j < K + 2:
                jmask[p, ck, j] = 1.0
    betabase = (NA * NA + np.arange(NB, dtype=np.float32)[:, None] * NA).astype(
        np.float32
    )
    xbase = (np.arange(NB, dtype=np.float32)[:, None] * NA).astype(np.float32)
    c.update(
        rhs_init=rhs_init, bdmask_in=bdmask, negsel_in=negsel, wmask_in=wmask,
        betabase_in=betabase, xbase_in=xbase, gselx_in=gselx, backsel_in=backsel,
        jmask_in=jmask, bsel16_in=bsel16, rjmask_in=rjmask,
    )
    return c


def kernel(X, y):
    X = np.ascontiguousarray(np.asarray(X, dtype=np.float32))
    y = np.ascontiguousarray(np.asarray(y, dtype=np.float32))
    B = X.shape[0]
    assert B == NCORES * NB and X.shape[1:] == (L, L) and y.shape == (B, L, 1)

    nc = _get_program()
    consts = _host_constants()

    d_mat = np.ascontiguousarray(
        np.concatenate([X[0], np.ones((L, 1), np.float32)], axis=1)
    )

    in_maps = []
    for c in range(NCORES):
        sl = slice(c * NB, (c + 1) * NB)
        y_t = np.ascontiguousarray(y[sl, :, 0].T)
        xt = np.ascontiguousarray(X[sl].transpose(0, 2, 1))          # [NB, A, L]
        xt_pad = np.concatenate(
            [xt, np.ones((NB, 1, L), np.float32)], axis=1
        ).reshape(NB * NA, L)
        m = {"d_mat": d_mat, "y_t": y_t, "xt_pad": np.ascontiguousarray(xt_pad)}
        m.update(consts)
        in_maps.append(m)

    res = run_bass_kernel_spmd(nc, in_maps, core_ids=list(range(NCORES)))
    out = np.concatenate([res.results[c]["out_r"] for c in range(NCORES)], axis=0)
    return out.reshape(B, L, 1).astype(np.float32)


def profile_once(X, y):
    """Run once with NTFF tracing; returns exec_time_ns (max across cores)."""
    X = np.ascontiguousarray(np.asarray(X, dtype=np.float32))
    y = np.ascontiguousarray(np.asarray(y, dtype=np.float32))
    nc = _get_program()
    consts = _host_constants()
    d_mat = np.ascontiguousarray(
        np.concatenate([X[0], np.ones((L, 1), np.float32)], axis=1)
    )
    in_maps = []
    for c in range(NCORES):
        sl = slice(c * NB, (c + 1) * NB)
        y_t = np.ascontiguousarray(y[sl, :, 0].T)
        xt = np.ascontiguousarray(X[sl].transpose(0, 2, 1))
        xt_pad = np.concatenate(
            [xt, np.ones((NB, 1, L), np.float32)], axis=1
        ).reshape(NB * NA, L)
        m = {"d_mat": d_mat, "y_t": y_t, "xt_pad": np.ascontiguousarray(xt_pad)}
        m.update(consts)
        in_maps.append(m)
    res = run_bass_kernel_spmd(
        nc, in_maps, core_ids=list(range(NCORES)), trace=True
    )
    return res.exec_time_ns


# revision 34
# speedup vs baseline: 1.1233x; 1.1164x over previous
"""Differentiable OMP (top-k masking) Trainium2 kernel.

Strategy (pure data parallelism over batch, 8 batches/core on 8 cores):
  The straight-through softmax terms cancel numerically in the forward pass,
  so each OMP iteration reduces to:
    pd    = proj0 - nzW @ G[idx_sel, :]        (argmax drive)
    idx_i = argmax |pd|
    solve (G[S,S] + reg I) nzW = proj0[S] incrementally (bordered inverse,
    rank-one product form) -- all O(i^2) work batched on 8 partitions.
  where G = D^T D (Gram of the shared dictionary) and proj0 = y @ D are
  computed once on device.  The final reconstruction gathers the 32 selected
  dictionary columns per batch from the X shard and combines with nzW on the
  tensor engine.  Only ~5 MB of the 32 MB X shard is ever read (indirect
  DMA gather with on-device indices).
"""

import os
import sys

for _p in ("/opt/trn_rl_repo", "/root/.axon_site/_ro/trn_rl_repo"):
    if os.path.isdir(_p) and _p not in sys.path:
        sys.path.insert(0, _p)

import numpy as np

import concourse.bass as bass
import concourse.mybir as mybir
import concourse.tile as tile
from concourse.bass_utils import run_bass_kernel_spmd
from concourse.masks import make_identity
from concourse.vector_clock import ScopedClock

F32 = mybir.dt.float32
U32 = mybir.dt.uint32
OP = mybir.AluOpType
AF = mybir.ActivationFunctionType
AX = mybir.AxisListType

NCORES = 8
NB = 8            # batches per core
L = 1024          # signal length
NA = 1025         # atoms (1024 + bias column)
K = 32            # n_nonzero_coefs
REG = float(np.log1p(np.exp(np.float32(-5.0), dtype=np.float32), dtype=np.float32))
GBUF_ROWS = NA + NB          # G rows then proj0 rows
NCHUNKS = [(0, 512), (512, 512), (1024, 1)]


_PATCHED = False


def _patch_tile_drain():
    """This walrus build rejects >1 sync waits per instruction: split the
    final-drain waits onto SP nops, and split any lowered instruction's
    extra waits onto same-engine nops."""
    global _PATCHED
    if _PATCHED:
        return
    _PATCHED = True

    _orig_commit_and_lower = tile.TileContext._commit_and_lower

    def _commit_and_lower_split(self, inst, original_block, old_bb_map, bb_to_exit):
        si = getattr(inst, "sync_info", None)
        if si is not None and si.on_wait and len(si.on_wait) > 1:
            waits = list(si.on_wait)
            for j, w in enumerate(waits[1:]):
                nop = mybir.InstNoOp(
                    name=f"{inst.name}-wsplit{j}", ins=[], outs=[], engine=inst.engine
                )
                nop.sync_info = mybir.SyncInfo(on_wait=[w], on_update=[])
                _orig_commit_and_lower(self, nop, original_block, old_bb_map, bb_to_exit)
            inst.sync_info = mybir.SyncInfo(
                on_wait=[waits[0]],
                on_update=list(si.on_update) if si.on_update else [],
            )
        return _orig_commit_and_lower(self, inst, original_block, old_bb_map, bb_to_exit)

    def _drain_and_barrier_split(self, tick_clock, wait_clock):
        nc = self.nc
        drain_inst = nc.sync.drain()
        wait_clock.add_sem_waits(
            drain_inst.ins, ScopedClock({None: tick_clock.global_clock})
        )
        si = drain_inst.ins.sync_info
        waits = list(si.on_wait) if si is not None and si.on_wait else []
        if len(waits) > 1:
            drain_inst.ins.sync_info = mybir.SyncInfo(
                on_wait=[waits[0]],
                on_update=list(si.on_update) if si.on_update else [],
            )
            for w in waits[1:]:
                n = nc.sync.nop()
                n.ins.sync_info = mybir.SyncInfo(on_wait=[w], on_update=[])

        nc.all_engine_barrier()
        assert self.sems is not None
        popped = nc._tile_sem_poison_stack.pop()
        assert popped is self._sem_poison
        nc.clear_and_free_semaphores(list(self.sems.allocated().values()))
        nc.all_engine_barrier()

    tile.TileContext._drain_and_barrier = _drain_and_barrier_split
    tile.TileContext._commit_and_lower = _commit_and_lower_split


def _cdiv(a, b):
    return (a + b - 1) // b


def _build_program():
    _patch_tile_drain()
    nc = bass.Bass()

    d_mat = nc.dram_tensor("d_mat", [L, NA], F32, kind="ExternalInput")
    y_t = nc.dram_tensor("y_t", [L, NB + 1], F32, kind="ExternalInput")
    xt_pad = nc.dram_tensor("xt_pad", [NB * NA, L], F32, kind="ExternalInput")
    rhs_init = nc.dram_tensor("rhs_init", [128, 3, NB], F32, kind="ExternalInput")
    bdmask_in = nc.dram_tensor("bdmask_in", [128, 3, NB, K], F32, kind="ExternalInput")
    negsel_in = nc.dram_tensor("negsel_in", [NB, 3, 128], F32, kind="ExternalInput")
    wmask_in = nc.dram_tensor("wmask_in", [128, 2, NB, K], F32, kind="ExternalInput")
    betabase_in = nc.dram_tensor("betabase_in", [NB, 1], F32, kind="ExternalInput")
    xbase_in = nc.dram_tensor("xbase_in", [NB, 1], F32, kind="ExternalInput")
    # partition-expansion helpers (indirect DMA wants one index per partition)
    gselx_in = nc.dram_tensor("gselx_in", [NB, 128], F32, kind="ExternalInput")
    backsel_in = nc.dram_tensor("backsel_in", [128, NB], F32, kind="ExternalInput")
    jmask_in = nc.dram_tensor("jmask_in", [128, 3, K + 2], F32, kind="ExternalInput")
    bsel16_in = nc.dram_tensor("bsel16_in", [NB, 128], F32, kind="ExternalInput")
    rjmask_in = nc.dram_tensor("rjmask_in", [128, 2, K], F32, kind="ExternalInput")
    out_r = nc.dram_tensor("out_r", [NB, L], F32, kind="ExternalOutput")
    gbuf = nc.dram_tensor("gbuf", [GBUF_ROWS * NA, 1], F32, kind="Internal")
    gall = nc.dram_tensor(
        "gall", [1024 * NA, 1], F32, kind="Internal", addr_space="Shared"
    )
    gpart = nc.dram_tensor("gpart", [128 * NA, 1], F32, kind="Internal")

    gflat_ap = gbuf[:, :]                                            # element gather
    grows_ap = gbuf[:, :].rearrange("(r c) x -> r (c x)", c=NA)      # row gather

    import contextlib

    with tile.TileContext(nc) as tc, contextlib.ExitStack() as ctx:
        st = ctx.enter_context(tc.tile_pool(name="st", bufs=1))
        pp = ctx.enter_context(tc.tile_pool(name="pp", bufs=1, space="PSUM"))

        # ---------------- persistent state ----------------
        dsb = st.tile([128, 8, NA], F32)          # D, L split in 8 chunks
        ysb = st.tile([128, 8, NB + 1], F32)      # y^T plus ones col
        dslice = st.tile([128, 8, 128], F32)      # this core's G row block of D
        gstage_s = st.tile([128, NA], F32)
        stage9 = st.tile([NB + 1, NA], F32)
        bigt = st.tile([128, 3, NA], F32)         # K-rows: proj0(8) + G rows
        rhs_t = st.tile([128, 3, NB], F32)        # matmul weights per K-row
        rhsi = st.tile([128, 3, NB], F32)         # inject-row pattern (ID8)
        rhsr = st.tile([128, NB], F32)            # rebuild scratch
        bdmask = st.tile([128, 3, NB, K], F32)
        wmask = st.tile([128, 2, NB, K], F32)
        negsel = st.tile([NB, 3, 128], F32)
        betabase = st.tile([NB, 1], F32)
        xbase = st.tile([NB, 1], F32)
        id128 = st.tile([128, 128], F32)
        gselx = st.tile([NB, 128], F32)
        backsel = st.tile([128, NB], F32)
        jmask = st.tile([128, 3, K + 2], F32)
        bsel16 = st.tile([NB, 128], F32)
        rjmask = st.tile([128, 2, K], F32)
        prodg = st.tile([128, K + 2], F32)
        fillr = st.tile([NB, 512], mybir.dt.bfloat16)   # HAM warm-keeper rhs
        prod2_t = st.tile([128, 3, NB, K], F32)
        rhsr2 = st.tile([128, 3, NB], F32)
        offsP = st.tile([128, 3], F32)
        offsPu = st.tile([128, 3], U32)
        gsmP = st.tile([128, 3], F32)
        grhs = st.tile([128, 3, K + 2], F32)
        xoffP = st.tile([128, 2], F32)
        xoffPu = st.tile([128, 2], U32)
        prodr = st.tile([128, K], F32)

        vmat = st.tile([NB, K, K], F32)
        nzw = st.tile([NB, K], F32)
        sinv_v = st.tile([NB, K], F32)
        bvec = st.tile([NB, K], F32)
        idxm = st.tile([NB, K], F32)
        gsm = st.tile([NB, K + 2], F32)
        offs = st.tile([NB, K + 2], F32)
        idxf = st.tile([NB, 1], F32)
        u_t = st.tile([NB, K], F32)
        c_t = st.tile([NB, K], F32)
        ct_t = st.tile([NB, K], F32)
        tmp3 = st.tile([NB, K, K], F32)
        tmp4 = st.tile([NB, K, K], F32)
        tmp5 = st.tile([NB, K], F32)
        sdot = st.tile([NB, 1], F32)
        s_t = st.tile([NB, 1], F32)
        alpha = st.tile([NB, 1], F32)
        ubdot = st.tile([NB, 1], F32)
        pdabs = st.tile([NB, NA], F32)
        mx8 = st.tile([NB, 8], F32)
        mi8 = st.tile([NB, 8], U32)
        prod_t = st.tile([128, NB, K], F32)
        xoff = st.tile([NB, K], F32)
        xsel = st.tile([128, 2, L], F32)
        wsel = st.tile([128, 2, NB], F32)
        outsb = st.tile([NB, L], F32)

        pdps = pp.tile([NB, NA], F32, tag="pdps")   # 3 PSUM banks
        m2ps = pp.tile([128, K + 2], F32, tag="m2")

        # ---------------- preamble: loads ----------------
        nc.sync.dma_start(
            out=dsb[:],
            in_=d_mat[:, :].rearrange("(i p) a -> p i a", p=128),
        )
        nc.sync.dma_start(
            out=ysb[:],
            in_=y_t[:, :].rearrange("(i p) b -> p i b", p=128),
        )
        pid = nc.sync.partition_id()
        nc.sync.dma_start(
            out=dslice[:],
            in_=d_mat[:, :].rearrange("(i p) a -> p i a", p=128)[
                :, :, bass.ds(pid * 128, 128)
            ],
        )
        nc.sync.dma_start(out=rhs_t[:], in_=rhs_init[:, :, :])
        nc.sync.dma_start(out=rhsi[:], in_=rhs_init[:, :, :])
        nc.sync.dma_start(out=bdmask[:], in_=bdmask_in[:, :, :, :])
        nc.sync.dma_start(out=wmask[:], in_=wmask_in[:, :, :, :])
        nc.sync.dma_start(out=negsel[:], in_=negsel_in[:, :, :])
        nc.sync.dma_start(out=betabase[:], in_=betabase_in[:, :])
        nc.sync.dma_start(out=xbase[:], in_=xbase_in[:, :])
        nc.sync.dma_start(out=gselx[:], in_=gselx_in[:, :])
        nc.sync.dma_start(out=backsel[:], in_=backsel_in[:, :])
        nc.sync.dma_start(out=jmask[:], in_=jmask_in[:, :, :])
        nc.sync.dma_start(out=bsel16[:], in_=bsel16_in[:, :])
        nc.sync.dma_start(out=rjmask[:], in_=rjmask_in[:, :, :])
        make_identity(nc, id128[:])

        nc.vector.memset(fillr[:], 0.0)
        nc.vector.memset(vmat[:], 0.0)
        nc.vector.memset(nzw[:], 0.0)
        nc.vector.memset(offs[:], 0.0)
        nc.vector.memset(gsmP[:], 0.0)

        _fill_pool = [pp]

        def warm_fill(dep_ap, tag="mx"):
            """Dummy bf16 matmul: keeps the PE HAM activity window busy so
            fp32 matmuls run at 2.4 GHz. Output is never read; the lhsT
            bitcast ties it to per-phase state so the scheduler spreads
            the fillers across the timeline."""
            fps = _fill_pool[0].tile([2, 512], F32, tag=tag)
            nc.tensor.matmul(
                fps[:],
                lhsT=dep_ap.bitcast(mybir.dt.bfloat16)[:, 0:2],
                rhs=fillr[:],
                start=True,
                stop=True,
            )

        # ---------------- G = D^T D (distributed) and proj0 ----------------
        # Core c computes G rows [128c, 128c+128) against the full D, writes
        # them to gpart, then an 8-core AllGather assembles rows 0..1023 in
        # gbuf. Row 1024 (the bias row, ones^T D) and proj0 ride along on a
        # single [9, 1025] matmul via the ones column in ysb.
        with tc.tile_pool(name="gp", bufs=2, space="PSUM") as gp:
            for (n0, nl) in NCHUNKS:
                gps = gp.tile([128, 512], F32, tag="gps")
                for kk in range(8):
                    nc.tensor.matmul(
                        gps[:, :nl],
                        lhsT=dslice[:, kk, :],
                        rhs=dsb[:, kk, n0 : n0 + nl],
                        start=(kk == 0),
                        stop=(kk == 7),
                    )
                nc.scalar.copy(out=gstage_s[:, n0 : n0 + nl], in_=gps[:, :nl])
                warm_fill(gstage_s[0:NB, n0 : n0 + 1], tag="gps")
            pj = pp.tile([NB + 1, NA], F32, tag="pdps")
            for (n0, nl) in NCHUNKS:
                for kk in range(8):
                    nc.tensor.matmul(
                        pj[:, n0 : n0 + nl],
                        lhsT=ysb[:, kk, :],
                        rhs=dsb[:, kk, n0 : n0 + nl],
                        start=(kk == 0),
                        stop=(kk == 7),
                    )
            nc.scalar.copy(out=stage9[:], in_=pj[:])
            nc.vector.tensor_copy(bigt[0:NB, 0, :], stage9[0:NB, :])

        nc.sync.dma_start(
            out=gpart[:, :].rearrange("(p c) x -> p (c x)", c=NA), in_=gstage_s[:]
        )
        nc.gpsimd.collective_compute(
            kind="AllGather",
            op=OP.bypass,
            replica_groups=[[0, 1, 2, 3, 4, 5, 6, 7]],
            ins=[gpart[:, :]],
            outs=[gall[:, :]],
        )
        # the shared buffer is one allocation across cores; copy to the
        # core-private gbuf so per-core proj0 rows don't clobber each other
        nc.sync.dma_start(
            out=gbuf[0 : 1024 * NA, :].rearrange("(r c) x -> r (c x)", c=NA),
            in_=gall[:, :].rearrange("(r c) x -> r (c x)", c=NA),
        )
        nc.sync.dma_start(out=grows_ap[1024:1025, :], in_=stage9[NB : NB + 1, :])

        # proj0 (batch-major) -> DRAM rows 1025..1032
        nc.sync.dma_start(out=grows_ap[NA : NA + NB, :], in_=stage9[0:NB, :])

        # ---------------- OMP iterations ----------------
        for i in range(K):
            rows = 8 + 8 * i
            nk = _cdiv(rows, 128)
            for ck in range(nk):
                cnt = min(128, rows - 128 * ck)
                for (n0, nl) in NCHUNKS:
                    nc.tensor.matmul(
                        pdps[:, n0 : n0 + nl],
                        lhsT=rhs_t[0:cnt, ck, :],
                        rhs=bigt[0:cnt, ck, n0 : n0 + nl],
                        start=(ck == 0),
                        stop=(ck == nk - 1),
                    )
            nc.scalar.activation(pdabs[:], pdps[:], AF.Abs)
            nc.vector.max(out=mx8[:], in_=pdabs[:])
            nc.vector.max_index(mi8[:], mx8[:], pdabs[:])
            warm_fill(pdabs[:, 0:1])
            warm_fill(mx8[:, 0:1])
            nc.vector.tensor_copy(idxf[:], mi8[:, 0:1])
            warm_fill(idxf[:, 0:1])
            nc.vector.tensor_copy(idxm[:, i : i + 1], idxf[:])

            # gather offsets: cols [0:i]=g, [i]=diag, [i+1]=beta
            if i > 0:
                nc.vector.scalar_tensor_tensor(
                    out=offs[:, 0:i],
                    in0=idxf[:].to_broadcast([NB, i]),
                    scalar=float(NA),
                    in1=idxm[:, 0:i],
                    op0=OP.mult,
                    op1=OP.add,
                )
            nc.vector.tensor_scalar_mul(offs[:, i : i + 1], idxf[:], float(NA + 1))
            nc.vector.tensor_scalar(
                out=offs[:, i + 1 : i + 2],
                in0=idxf[:],
                scalar1=betabase[:],
                scalar2=None,
                op0=OP.add,
            )
            # hw indirect DMA gathers one index per destination partition:
            # expand offs [8, j] -> partition-major rows r = 8j + b via matmul,
            # gather one element per partition, then collapse back to [8, j].
            nitem = 8 * (i + 2)
            nkg = _cdiv(nitem, 128)
            gsmps = pp.tile([NB, K + 2], F32, tag="mx")
            nc.tensor.matmul(
                m2ps[:], lhsT=gselx[:], rhs=offs[:], start=True, stop=True
            )
            for ck in range(nkg):
                cntg = min(128, nitem - 128 * ck)
                nc.vector.tensor_tensor(
                    out=prodg[:], in0=m2ps[:], in1=jmask[:, ck, :], op=OP.mult
                )
                nc.vector.tensor_reduce(
                    out=offsP[:, ck : ck + 1], in_=prodg[:], axis=AX.X, op=OP.add
                )
                nc.vector.tensor_copy(
                    offsPu[:, ck : ck + 1], offsP[:, ck : ck + 1]
                )
                nc.gpsimd.indirect_dma_start(
                    out=gsmP[0:cntg, ck : ck + 1],
                    out_offset=None,
                    in_=gflat_ap,
                    in_offset=bass.IndirectOffsetOnAxis(
                        ap=offsPu[0:cntg, ck : ck + 1], axis=0
                    ),
                )
                warm_fill(offsP[0:NB, ck : ck + 1])
                nc.vector.tensor_scalar(
                    out=grhs[:, ck, :], in0=jmask[:, ck, :],
                    scalar1=gsmP[:, ck : ck + 1], scalar2=None, op0=OP.mult,
                )
                nc.tensor.matmul(
                    gsmps[:], lhsT=backsel[:], rhs=grhs[:, ck, :],
                    start=(ck == 0), stop=(ck == nkg - 1),
                )
            nc.vector.tensor_copy(gsm[:], gsmps[:])
            warm_fill(gsm[:, 0:1])
            if i < K - 1:
                r0 = 8 + 8 * i
                gck, gp0 = r0 // 128, r0 % 128
                nc.gpsimd.indirect_dma_start(
                    out=bigt[gp0 : gp0 + 8, gck, :],
                    out_offset=None,
                    in_=grows_ap,
                    in_offset=bass.IndirectOffsetOnAxis(ap=mi8[:, 0:1], axis=0),
                )

            d_ap = gsm[:, i : i + 1]
            b_ap = gsm[:, i + 1 : i + 2]
            if i == 0:
                nc.vector.tensor_scalar_add(s_t[:], d_ap, REG)
                nc.vector.reciprocal(sinv_v[:, 0:1], s_t[:])
                nc.vector.tensor_copy(bvec[:, 0:1], b_ap)
                nc.vector.scalar_tensor_tensor(
                    out=alpha[:], in0=b_ap, scalar=-1.0,
                    in1=sinv_v[:, 0:1], op0=OP.mult, op1=OP.mult,
                )
                nc.vector.tensor_scalar_mul(nzw[:, 0:1], alpha[:], -1.0)
                nc.vector.memset(vmat[:, 0:1, 0:1], -1.0)
            else:
                g_ap = gsm[:, 0:i]
                nc.vector.tensor_tensor(
                    out=tmp3[:, 0:i, 0:i],
                    in0=vmat[:, 0:i, 0:i],
                    in1=g_ap.unsqueeze(1).to_broadcast([NB, i, i]),
                    op=OP.mult,
                )
                nc.vector.tensor_reduce(
                    out=c_t[:, 0:i], in_=tmp3[:, 0:i, 0:i], axis=AX.X, op=OP.add
                )
                nc.vector.tensor_tensor(
                    out=ct_t[:, 0:i], in0=c_t[:, 0:i], in1=sinv_v[:, 0:i], op=OP.mult
                )
                nc.vector.tensor_tensor(
                    out=tmp4[:, 0:i, 0:i],
                    in0=vmat[:, 0:i, 0:i].transpose([0, 2, 1]),
                    in1=ct_t[:, 0:i].unsqueeze(1).to_broadcast([NB, i, i]),
                    op=OP.mult,
                )
                nc.vector.tensor_reduce(
                    out=u_t[:, 0:i], in_=tmp4[:, 0:i, 0:i], axis=AX.X, op=OP.add
                )
                warm_fill(u_t[:, 0:1])
                nc.vector.tensor_tensor(
                    out=tmp5[:, 0:i], in0=g_ap, in1=u_t[:, 0:i], op=OP.mult
                )
                nc.vector.tensor_reduce(
                    out=sdot[:], in_=tmp5[:, 0:i], axis=AX.X, op=OP.add
                )
                nc.vector.scalar_tensor_tensor(
                    out=s_t[:], in0=d_ap, scalar=REG, in1=sdot[:],
                    op0=OP.add, op1=OP.subtract,
                )
                nc.vector.reciprocal(sinv_v[:, i : i + 1], s_t[:])
                nc.vector.tensor_tensor(
                    out=tmp5[:, 0:i], in0=u_t[:, 0:i], in1=bvec[:, 0:i], op=OP.mult
                )
                nc.vector.tensor_reduce(
                    out=ubdot[:], in_=tmp5[:, 0:i], axis=AX.X, op=OP.add
                )
                nc.vector.scalar_tensor_tensor(
                    out=alpha[:], in0=ubdot[:], scalar=b_ap,
                    in1=sinv_v[:, i : i + 1], op0=OP.subtract, op1=OP.mult,
                )
                nc.vector.scalar_tensor_tensor(
                    out=nzw[:, 0:i], in0=u_t[:, 0:i], scalar=alpha[:],
                    in1=nzw[:, 0:i], op0=OP.mult, op1=OP.add,
                )
                nc.vector.tensor_scalar_mul(nzw[:, i : i + 1], alpha[:], -1.0)
                nc.vector.tensor_copy(vmat[:, i, 0:i], u_t[:, 0:i])
                nc.vector.memset(vmat[:, i : i + 1, i : i + 1], -1.0)
                nc.vector.tensor_copy(bvec[:, i : i + 1], b_ap)
                warm_fill(alpha[:, 0:1])

            # rebuild matmul weights (rows 8..8+8(i+1)) for next iteration
            if i < K - 1:
                rows_next = 8 + 8 * (i + 1)
                nk2 = _cdiv(rows_next, 128)
                # M1 expansion differs per chunk only through bdmask; one
                # matmul + one fused tt/reduce/add across all live chunks.
                m1ps = pp.tile([128, K], F32, tag="mx")
                nc.tensor.matmul(
                    m1ps[:], lhsT=negsel[:, 0, :], rhs=nzw[:],
                    start=True, stop=True,
                )
                nc.vector.tensor_tensor(
                    out=prod2_t[:, 0:nk2, :, :],
                    in0=m1ps[:]
                    .unsqueeze(1)
                    .unsqueeze(1)
                    .to_broadcast([128, nk2, NB, K]),
                    in1=bdmask[:, 0:nk2, :, :],
                    op=OP.mult,
                )
                nc.vector.tensor_reduce(
                    out=rhsr2[:, 0:nk2, :], in_=prod2_t[:, 0:nk2, :, :],
                    axis=AX.X, op=OP.add,
                )
                nc.vector.tensor_tensor(
                    out=rhs_t[:, 0:nk2, :], in0=rhsr2[:, 0:nk2, :],
                    in1=rhsi[:, 0:nk2, :], op=OP.add,
                )
                warm_fill(rhsr2[0:NB, 0, 0:1])

        # ---------------- reconstruction ----------------
        # row r = 128*ck + p of the gather maps to (b = p//16, k = 16ck + p%16)
        nc.vector.tensor_scalar(
            out=xoff[:], in0=idxm[:], scalar1=xbase[:], scalar2=None, op0=OP.add
        )
        xt_rows = xt_pad[:, :]
        nc.tensor.matmul(
            m2ps[:, 0:K], lhsT=bsel16[:], rhs=xoff[:], start=True, stop=True
        )
        for ck in range(2):
            m1ps = pp.tile([128, K], F32, tag="mx")
            nc.vector.tensor_tensor(
                out=prodr[:], in0=m2ps[:, 0:K], in1=rjmask[:, ck, :], op=OP.mult
            )
            nc.vector.tensor_reduce(
                out=xoffP[:, ck : ck + 1], in_=prodr[:], axis=AX.X, op=OP.add
            )
            nc.vector.tensor_copy(xoffPu[:, ck : ck + 1], xoffP[:, ck : ck + 1])
            nc.gpsimd.indirect_dma_start(
                out=xsel[:, ck, :],
                out_offset=None,
                in_=xt_rows,
                in_offset=bass.IndirectOffsetOnAxis(
                    ap=xoffPu[:, ck : ck + 1], axis=0
                ),
            )
            nc.tensor.matmul(
                m1ps[:], lhsT=bsel16[:], rhs=nzw[:], start=True, stop=True
            )
            nc.vector.tensor_tensor(
                out=prod_t[:, :, :],
                in0=m1ps[:, 0:K].unsqueeze(1).to_broadcast([128, NB, K]),
                in1=wmask[:, ck, :, :],
                op=OP.mult,
            )
            nc.vector.tensor_reduce(
                out=wsel[:, ck, :], in_=prod_t[:, :, :], axis=AX.X, op=OP.add
            )
        ops = pp.tile([NB, L], F32, tag="pdps")
        for ck in range(2):
            for (n0, nl) in [(0, 512), (512, 512)]:
                nc.tensor.matmul(
                    ops[:, n0 : n0 + nl],
                    lhsT=wsel[:, ck, :],
                    rhs=xsel[:, ck, n0 : n0 + nl],
                    start=(ck == 0),
                    stop=(ck == 1),
                )
        nc.scalar.copy(out=outsb[:], in_=ops[:])
        nc.sync.dma_start(out=out_r[:, :], in_=outsb[:])

    return nc


_NC_CACHE = None


def _get_program():
    global _NC_CACHE
    if _NC_CACHE is None:
        _NC_CACHE = _build_program()
    return _NC_CACHE


def _host_constants():
    c = {}
    rhs_init = np.zeros((128, 3, NB), np.float32)
    for b in range(NB):
        rhs_init[b, 0, b] = 1.0
    bdmask = np.zeros((128, 3, NB, K), np.float32)
    negsel = np.zeros((NB, 3, 128), np.float32)
    for ck in range(3):
        for p in range(128):
            negsel[p % 8, ck, p] = -1.0     # validity filtering lives in bdmask
            r = ck * 128 + p
            if r < 8 or r >= 8 + 8 * K:
                continue
            b, kk = (r - 8) % 8, (r - 8) // 8
            bdmask[p, ck, b, kk] = 1.0
    wmask = np.zeros((128, 2, NB, K), np.float32)
    bsel16 = np.zeros((NB, 128), np.float32)
    rjmask = np.zeros((128, 2, K), np.float32)
    for p in range(128):
        bsel16[p // 16, p] = 1.0
        for ck in range(2):
            b, kk = p // 16, ck * 16 + p % 16
            wmask[p, ck, b, kk] = 1.0
            rjmask[p, ck, kk] = 1.0
    gselx = np.zeros((NB, 128), np.float32)
    backsel = np.zeros((128, NB), np.float32)
    jmask = np.zeros((128, 3, K + 2), np.float32)
    for p in range(128):
        gselx[p % 8, p] = 1.0
        backsel[p, p % 8] = 1.0
        for ck in range(3):
            j = 16 * ck + p // 8
            if j < K + 2:
                jmask[p, ck, j] = 1.0
    betabase = (NA * NA + np.arange(NB, dtype=np.float32)[:, None] * NA).astype(
        np.float32
    )
    xbase = (np.arange(NB, dtype=np.float32)[:, None] * NA).astype(np.float32)
    c.update(
        rhs_init=rhs_init, bdmask_in=bdmask, negsel_in=negsel, wmask_in=wmask,
        betabase_in=betabase, xbase_in=xbase, gselx_in=gselx, backsel_in=backsel,
        jmask_in=jmask, bsel16_in=bsel16, rjmask_in=rjmask,
    )
    return c


def kernel(X, y):
    X = np.ascontiguousarray(np.asarray(X, dtype=np.float32))
    y = np.ascontiguousarray(np.asarray(y, dtype=np.float32))
    B = X.shape[0]
    assert B == NCORES * NB and X.shape[1:] == (L, L) and y.shape == (B, L, 1)

    nc = _get_program()
    consts = _host_constants()

    d_mat = np.ascontiguousarray(
        np.concatenate([X[0], np.ones((L, 1), np.float32)], axis=1)
    )

    in_maps = []
    for c in range(NCORES):
        sl = slice(c * NB, (c + 1) * NB)
        y_t = np.ascontiguousarray(
            np.concatenate([y[sl, :, 0].T, np.ones((L, 1), np.float32)], axis=1)
        )
        xt = np.ascontiguousarray(X[sl].transpose(0, 2, 1))          # [NB, A, L]
        xt_pad = np.concatenate(
            [xt, np.ones((NB, 1, L), np.float32)], axis=1
        ).reshape(NB * NA, L)
        m = {"d_mat": d_mat, "y_t": y_t, "xt_pad": np.ascontiguousarray(xt_pad)}
        m.update(consts)
        in_maps.append(m)

    res = run_bass_kernel_spmd(nc, in_maps, core_ids=list(range(NCORES)))
    out = np.concatenate([res.results[c]["out_r"] for c in range(NCORES)], axis=0)
    return out.reshape(B, L, 1).astype(np.float32)


def profile_once(X, y):
    """Run once with NTFF tracing; returns exec_time_ns (max across cores)."""
    X = np.ascontiguousarray(np.asarray(X, dtype=np.float32))
    y = np.ascontiguousarray(np.asarray(y, dtype=np.float32))
    nc = _get_program()
    consts = _host_constants()
    d_mat = np.ascontiguousarray(
        np.concatenate([X[0], np.ones((L, 1), np.float32)], axis=1)
    )
    in_maps = []
    for c in range(NCORES):
        sl = slice(c * NB, (c + 1) * NB)
        y_t = np.ascontiguousarray(
            np.concatenate([y[sl, :, 0].T, np.ones((L, 1), np.float32)], axis=1)
        )
        xt = np.ascontiguousarray(X[sl].transpose(0, 2, 1))
        xt_pad = np.concatenate(
            [xt, np.ones((NB, 1, L), np.float32)], axis=1
        ).reshape(NB * NA, L)
        m = {"d_mat": d_mat, "y_t": y_t, "xt_pad": np.ascontiguousarray(xt_pad)}
        m.update(consts)
        in_maps.append(m)
    res = run_bass_kernel_spmd(
        nc, in_maps, core_ids=list(range(NCORES)), trace=True
    )
    return res.exec_time_ns
